# revision 27
# baseline (speedup 1.0000x reference)
"""AttnBlock (GroupNorm -> QKV 1x1 -> full self-attention over 4096 tokens ->
out-proj -> residual) for Trainium2, 8 NeuronCores.

Sharding: batch b in {0..3} x sequence-half h in {0,1} -> core = 2*b + h.
Each core gets its batch's full x (columns rotated so its own 2048 query
columns come first), computes GroupNorm + V for all 4096 positions, and
attention + output projection for its own 2048 query rows.

Math tricks:
- K is never materialized: S = (Wk^T (Wq h + bq))^T h. The host pre-folds
  W' = Wq^T Wk and qtb = Wk^T bq (float64), so Qt = W'^T.T h + qtb needs one
  device matmul. The bk term of S is constant along the softmax axis and
  drops out.
- VT [m, c] is produced directly as matmul(lhsT=H-block, rhs=Wv^T) — no
  transposes on the V path.
- Softmax runs without max subtraction (logits for this problem are ~N(0,1),
  |S| < 10; exp stays far inside fp32/bf16 range).

Precision: Qt/out-proj in f32r (TF32-like), big attention matmuls in bf16
with fp32 PSUM accumulation, softmax exp in fp32 on ScalarE.
"""

import math

import numpy as np
import ml_dtypes

import concourse.bass as bass
import concourse.tile as tile
from concourse import bacc, mybir
from concourse import bass_utils
from concourse.masks import make_identity

F32 = mybir.dt.float32
F32R = mybir.dt.float32r
BF16 = mybir.dt.bfloat16
AF = mybir.ActivationFunctionType
ALU = mybir.AluOpType
AX = mybir.AxisListType

P = 128
C = 512          # channels
N = 4096         # h*w
NOWN = 2048      # query columns owned per core
CO = C // P      # 4 channel blocks
NT = N // 512    # 8 column tiles
NQT = NOWN // 512  # 4 own column tiles
QB = NOWN // P   # 16 query row blocks
MB = N // P      # 32 key blocks
FD = 512
SCALE = 1.0 / math.sqrt(C)
EPS = 1e-6
N_CORES = 8


def build_nc(loop_reps=None, qb_limit=QB, do_c=True):
    nc = bacc.Bacc("TRN2", target_bir_lowering=False, debug=False,
                   num_devices=N_CORES)
    d = {}
    d["xb"] = nc.dram_tensor("xb", [C, N], F32, kind="ExternalInput").ap()
    d["wqtk"] = nc.dram_tensor("wqtk", [C, C], BF16, kind="ExternalInput").ap()
    d["wvtb"] = nc.dram_tensor("wvtb", [C, C], BF16, kind="ExternalInput").ap()
    for v in ("gamma", "beta", "qtb", "bv", "bo"):
        d[v] = nc.dram_tensor(v, [C], F32, kind="ExternalInput").ap()
    d["mask"] = nc.dram_tensor("mask", [P, 8], F32, kind="ExternalInput").ap()
    d["maskt"] = nc.dram_tensor("maskt", [P, P], F32, kind="ExternalInput").ap()
    y = nc.dram_tensor("y", [C, NOWN], F32, kind="ExternalOutput").ap()

    xr = d["xb"].rearrange("(co ci) n -> ci co n", ci=P)
    yr = y.rearrange("(oo oi) n -> oi oo n", oi=P)

    with tile.TileContext(nc) as tc:
        with (
            tc.tile_pool(name="big", bufs=1) as big,
            tc.tile_pool(name="wp", bufs=1) as wp,
            tc.tile_pool(name="pp", bufs=2) as pp,
            tc.tile_pool(name="qs", bufs=4) as qs,
            tc.tile_pool(name="small", bufs=3) as small,
            tc.tile_pool(name="pt", bufs=2) as ptp,
            tc.tile_pool(name="single", bufs=1) as single,
            tc.tile_pool(name="ps", bufs=4, space="PSUM") as ps,
            tc.tile_pool(name="ps2", bufs=2, space="PSUM") as ps2,
            tc.tile_pool(name="pst", bufs=2, space="PSUM") as pst,
        ):
            # ---------- constants ----------
            ident32s = single.tile([P, P], F32, tag="id32s")
            make_identity(nc, ident32s[:])
            ident32 = single.tile([P, P], F32, tag="id32")
            nc.vector.tensor_copy(ident32[:], ident32s[:])
            ident16 = single.tile([P, P], BF16, tag="id16")
            nc.vector.tensor_copy(ident16[:], ident32s[:])
            eps_t = single.tile([P, 1], F32, tag="eps")
            nc.vector.memset(eps_t[:], EPS)
            mask_sb = single.tile([P, 8], F32, tag="mask")
            nc.sync.dma_start(mask_sb[:], d["mask"])
            maskt_sb = single.tile([P, P], F32, tag="maskt")
            nc.sync.dma_start(maskt_sb[:], d["maskt"])
            vec_sb = {}
            for v in ("gamma", "beta", "qtb", "bo"):
                t = single.tile([P, CO], F32, tag=v)
                nc.sync.dma_start(t[:], d[v].rearrange("(co ci) -> ci co", ci=P))
                vec_sb[v] = t
            ones16 = single.tile([P, 1], BF16, tag="ones16")
            nc.vector.memset(ones16[:], 1.0)
            bv_row = single.tile([P, C], F32, tag="bvrow")
            bv_bcast = bass.AP(tensor=d["bv"].tensor, offset=d["bv"].offset,
                               ap=[[0, P], d["bv"].ap[0]])
            nc.sync.dma_start(bv_row[:], bv_bcast)

            # ---------- weights ----------
            wqtk_sb = wp.tile([P, CO, C], BF16, tag="wqtk")
            nc.sync.dma_start(wqtk_sb[:], d["wqtk"].rearrange(
                "(ko ki) c -> ki ko c", ki=P))
            wvtb_sb = wp.tile([P, CO, C], BF16, tag="wvtb")
            nc.sync.dma_start(wvtb_sb[:], d["wvtb"].rearrange(
                "(ko ki) c -> ki ko c", ki=P))
            # ---------- big persistent tensors ----------
            X = big.tile([P, CO, N], F32, tag="X")         # raw x resident
            H = big.tile([P, CO, N], BF16, tag="H")        # normalized x
            VT = big.tile([P, MB, C], BF16, tag="VT")      # V transposed [m, c]
            QT = big.tile([P, CO, NOWN], BF16, tag="QT")   # scaled Qt [c, n]

            import contextlib
            loop_ctx = (tc.For_i(0, loop_reps, 1) if loop_reps
                        else contextlib.nullcontext())
            loop_ctx.__enter__()

            # ---------- phase A: GroupNorm statistics ----------
            stats_all = single.tile([P, NT, CO, 6], F32, tag="stats")
            mvt = single.tile([P, CO, 2], F32, tag="mvt")
            stats8 = single.tile([P, 8], F32, tag="stats8")
            scr = single.tile([P, 12], F32, tag="scr")
            vals = single.tile([P, 8], F32, tag="vals")
            bc = single.tile([P, 8], F32, tag="bc")
            a_t = single.tile([P, CO], F32, tag="a_t")
            b_t = single.tile([P, CO], F32, tag="b_t")

            for t in range(NT):
                eng = nc.sync if t % 2 == 0 else nc.gpsimd
                eng.dma_start(X[:, :, t * FD:(t + 1) * FD],
                              xr[:, :, t * FD:(t + 1) * FD])
                for co in range(CO):
                    nc.vector.bn_stats(out=stats_all[:, t, co, :],
                                       in_=X[:, co, t * FD:(t + 1) * FD])
            for co in range(CO):
                nc.vector.bn_aggr(out=mvt[:, co, :], in_=stats_all[:, :, co, :])
            # stats8: cols 0:4 per-(ci,co) mean, cols 4:8 mean^2+var
            nc.vector.tensor_copy(stats8[:, 0:4], mvt[:, :, 0])
            nc.vector.tensor_tensor(stats8[:, 4:8], mvt[:, :, 0], mvt[:, :, 0],
                                    ALU.mult)
            nc.vector.tensor_tensor(stats8[:, 4:8], stats8[:, 4:8], mvt[:, :, 1],
                                    ALU.add)
            ps_s = pst.tile([P, 8], F32, tag="tr")
            nc.tensor.matmul(ps_s[:8, :], mask_sb[:], stats8[:], start=True,
                             stop=True)
            nc.vector.tensor_scalar_mul(scr[:8, 0:4], ps_s[:8, 0:4], 1.0 / 16)
            nc.vector.tensor_scalar_mul(scr[:8, 4:8], ps_s[:8, 4:8], 1.0 / 16)
            nc.vector.tensor_tensor(scr[:8, 8:12], scr[:8, 0:4], scr[:8, 0:4],
                                    ALU.mult)
            nc.vector.tensor_tensor(scr[:8, 4:8], scr[:8, 4:8], scr[:8, 8:12],
                                    ALU.subtract)
            nc.scalar.activation(out=scr[:8, 4:8], in_=scr[:8, 4:8],
                                 func=AF.Sqrt, bias=eps_t[:8], scale=1.0)
            nc.vector.reciprocal(out=scr[:8, 4:8], in_=scr[:8, 4:8])
            nc.vector.memset(vals[:], 0.0)
            nc.vector.tensor_copy(vals[:8, :], scr[:8, 0:8])
            ps_b = pst.tile([P, 8], F32, tag="tr")
            nc.tensor.matmul(ps_b[:], maskt_sb[:], vals[:], start=True,
                             stop=True)
            nc.vector.tensor_copy(bc[:], ps_b[:])
            # a = rstd*gamma ; b = beta - mean*a
            nc.vector.tensor_tensor(a_t[:], bc[:, 4:8], vec_sb["gamma"][:],
                                    ALU.mult)
            nc.vector.tensor_tensor(b_t[:], bc[:, 0:4], a_t[:], ALU.mult)
            nc.vector.tensor_tensor(b_t[:], vec_sb["beta"][:], b_t[:],
                                    ALU.subtract)

            # ---------- phase B: normalize + Qt projection ----------
            for t in range(NT):
                for co in range(CO):
                    nc.vector.tensor_scalar(
                        out=H[:, co, t * FD:(t + 1) * FD],
                        in0=X[:, co, t * FD:(t + 1) * FD],
                        scalar1=a_t[:, co:co + 1], scalar2=b_t[:, co:co + 1],
                        op0=ALU.mult, op1=ALU.add)
                if t < NQT:
                    for cb in range(CO):
                        ps_qt = ps2.tile([P, FD], F32, tag="mm512")
                        for co in range(CO):
                            nc.tensor.matmul(ps_qt[:],
                                             wqtk_sb[:, co, cb * P:(cb + 1) * P],
                                             H[:, co, t * FD:(t + 1) * FD],
                                             start=(co == 0),
                                             stop=(co == CO - 1))
                        nc.vector.tensor_scalar(
                            out=QT[:, cb, t * FD:(t + 1) * FD],
                            in0=ps_qt[:],
                            scalar1=vec_sb["qtb"][:, cb:cb + 1], scalar2=SCALE,
                            op0=ALU.add, op1=ALU.mult)
                # VT blocks for this tile's columns (only need H[t])
                for mb in range(t * 4, t * 4 + 4):
                    ps_vt = ps2.tile([P, FD], F32, tag="mm512")
                    for co in range(CO):
                        nc.tensor.matmul(ps_vt[:],
                                         H[:, co, mb * P:(mb + 1) * P],
                                         wvtb_sb[:, co, :], start=(co == 0),
                                         stop=(co == CO - 1))
                    nc.vector.tensor_tensor(VT[:, mb, :], ps_vt[:],
                                            bv_row[:], ALU.add)

            # residual pre-bias: X += bo (per-channel), used by the epilogue
            for co in range(CO):
                nc.vector.tensor_scalar_add(X[:, co, :], X[:, co, :],
                                            vec_sb["bo"][:, co:co + 1])

            # ---------- phase C: attention over qb-pairs, S computed ----------
            # transposed: ps = H_block^T @ QT_pair -> S^T [m128, n256]; exp
            # writes P^T straight to SBUF (no transposes); row sums via a
            # ones-matmul over the m partitions.
            def emit_s_block(pp_):
                PT2 = ptp.tile([P, MB, 2 * P], BF16, tag="pt")
                st = qs.tile([P, 12], F32, tag="qstats")
                for mb in range(MB):
                    ps_st = ps.tile([P, 2 * P], F32, tag="st256")
                    for co in range(CO):
                        nc.tensor.matmul(ps_st[:],
                                         H[:, co, mb * P:(mb + 1) * P],
                                         QT[:, co, pp_ * 2 * P:
                                            (pp_ + 1) * 2 * P],
                                         start=(co == 0), stop=(co == CO - 1))
                    nc.scalar.activation(out=PT2[:, mb, :], in_=ps_st[:],
                                         func=AF.Exp, bias=0.0, scale=1.0)
                for h in range(2):
                    ps_rs = pst.tile([P, 8], F32, tag="tr")
                    for mb in range(MB):
                        nc.tensor.matmul(ps_rs[:, 0:1],
                                         PT2[:, mb, h * P:(h + 1) * P],
                                         ones16[:], start=(mb == 0),
                                         stop=(mb == MB - 1))
                    nc.vector.reciprocal(out=st[:, h:h + 1], in_=ps_rs[:, 0:1])
                return PT2, st

            def emit_av_block(pp_, PT2, st):
                for h in range(2):
                    qb = 2 * pp_ + h
                    ps_o = ps2.tile([P, FD], F32, tag="mm512")
                    for mb in range(MB):
                        nc.tensor.matmul(ps_o[:], PT2[:, mb, h * P:(h + 1) * P],
                                         VT[:, mb, :],
                                         start=(mb == 0), stop=(mb == MB - 1))
                    strip = small.tile([P, FD], F32, tag="strip")
                    nc.vector.tensor_scalar_mul(strip[:], ps_o[:],
                                                st[:, h:h + 1])
                    ps_ot = pst.tile([P, CO, P], F32, tag="tr")
                    for cb in range(CO):
                        nc.tensor.transpose(ps_ot[:, cb, :],
                                            strip[:, cb * P:(cb + 1) * P],
                                            ident32[:])
                    y_sb = small.tile([P, CO, P], F32, tag="ysb")
                    nc.vector.tensor_tensor(y_sb[:], ps_ot[:],
                                            X[:, :, qb * P:(qb + 1) * P],
                                            ALU.add)
                    nc.gpsimd.dma_start(yr[:, :, qb * P:(qb + 1) * P], y_sb[:])

            npair = (qb_limit if do_c else 0) // 2
            pending = None
            for pp_ in range(npair):
                blk = emit_s_block(pp_)
                if pending is not None:
                    emit_av_block(pp_ - 1, *pending)
                pending = blk
            if pending is not None:
                emit_av_block(npair - 1, *pending)
            loop_ctx.__exit__(None, None, None)

    nc.compile()
    return nc


_NC = None


def _get_nc():
    global _NC
    if _NC is None:
        _NC = build_nc()
    return _NC


def make_in_maps(inputs):
    x = np.asarray(inputs["x"], dtype=np.float32)
    wq = np.asarray(inputs["wq"], np.float64)
    wk = np.asarray(inputs["wk"], np.float64)
    wqtk = np.ascontiguousarray(
        (wq.T @ wk).astype(np.float32)).astype(ml_dtypes.bfloat16)
    qtb = (wk.T @ np.asarray(inputs["bq"], np.float64)).astype(np.float32)
    wv = np.asarray(inputs["wv"], np.float64)
    wo = np.asarray(inputs["wo"], np.float64)
    wvp = wo @ wv
    wvtb = np.ascontiguousarray(wvp.T.astype(np.float32)).astype(
        ml_dtypes.bfloat16)
    bvp = (wo @ np.asarray(inputs["bv"], np.float64)).astype(np.float32)
    gamma = np.asarray(inputs["gamma"], np.float32)
    beta = np.asarray(inputs["beta"], np.float32)
    bv = np.asarray(inputs["bv"], np.float32)
    bo = np.asarray(inputs["bo"], np.float32)
    mask = np.zeros((P, 8), np.float32)
    for ci in range(P):
        mask[ci, ci // 16] = 1.0
    maskt = np.zeros((P, P), np.float32)
    maskt[:8, :] = mask.T
    in_maps = []
    for core in range(N_CORES):
        b, h = core // 2, core % 2
        xb = x[b].reshape(C, N)
        xb_rot = np.ascontiguousarray(np.roll(xb, -NOWN * h, axis=1))
        in_maps.append({
            "xb": xb_rot, "wqtk": wqtk, "wvtb": wvtb,
            "gamma": gamma, "beta": beta, "qtb": qtb, "bv": bvp, "bo": bo,
            "mask": mask, "maskt": maskt,
        })
    return in_maps


def assemble(results, x_shape):
    B, C_, Hh, Ww = x_shape
    out = np.empty((B, C_, Hh * Ww), np.float32)
    for core in range(N_CORES):
        b, h = core // 2, core % 2
        out[b][:, NOWN * h:NOWN * (h + 1)] = results[core]["y"]
    return out.reshape(B, C_, Hh, Ww)


_EXEC = None


def _get_exec():
    """Build the jitted 8-core executor once per process."""
    global _EXEC
    if _EXEC is None:
        import jax
        from jax.experimental.shard_map import shard_map
        from jax.sharding import Mesh, PartitionSpec
        from concourse import bass2jax as b2j

        nc = _get_nc()
        b2j.install_neuronx_cc_hook()
        partition_name = (nc.partition_id_tensor.name
                          if nc.partition_id_tensor else None)
        in_names, out_names, out_avals, out_shapes = [], [], [], []
        for alloc in nc.m.functions[0].allocations:
            if not isinstance(alloc, mybir.MemoryLocationSet):
                continue
            name = alloc.memorylocations[0].name
            if alloc.kind == "ExternalInput":
                if name != partition_name:
                    in_names.append(name)
            elif alloc.kind == "ExternalOutput":
                out_names.append(name)
                shape = tuple(alloc.tensor_shape)
                dtype = mybir.dt.np(alloc.dtype)
                out_avals.append(jax.core.ShapedArray(shape, dtype))
                out_shapes.append((shape, dtype))
        all_names = tuple(in_names + out_names)
        if partition_name is not None:
            all_names = all_names + (partition_name,)

        def _body(*args):
            operands = list(args)
            if partition_name is not None:
                operands.append(b2j.partition_id_tensor())
            outs = b2j._bass_exec_p.bind(
                *operands, out_avals=tuple(out_avals), in_names=all_names,
                out_names=tuple(out_names), lowering_input_output_aliases=(),
                sim_require_finite=True, sim_require_nnan=True, nc=nc)
            return tuple(outs)

        devices = jax.devices()[:N_CORES]
        mesh = Mesh(np.asarray(devices), ("core",))
        nin = len(in_names) + len(out_names)
        fn = jax.jit(shard_map(_body, mesh=mesh,
                               in_specs=(PartitionSpec("core"),) * nin,
                               out_specs=(PartitionSpec("core"),) *
                               len(out_names),
                               check_rep=False),
                     keep_unused=True)
        _EXEC = (fn, in_names, out_names, out_shapes)
    return _EXEC


def kernel(**inputs) -> np.ndarray:
    fn, in_names, out_names, out_shapes = _get_exec()
    in_maps = make_in_maps(inputs)
    args = [np.concatenate([np.asarray(in_maps[c][nm]) for c in
                            range(N_CORES)], axis=0) for nm in in_names]
    args += [np.zeros((shape[0] * N_CORES,) + shape[1:], dtype)
             for shape, dtype in out_shapes]
    outs = fn(*args)
    yfull = np.asarray(outs[out_names.index("y")])
    results = [{"y": yfull[c * C:(c + 1) * C]} for c in range(N_CORES)]
    return assemble(results, np.asarray(inputs["x"]).shape)


def make_runner(nc, in_maps, reps=1):
    """Persistent jitted executor with device-resident inputs, for timing and
    low-overhead repeat runs."""
    import jax
    from jax.experimental.shard_map import shard_map
    from jax.sharding import Mesh, PartitionSpec, NamedSharding
    from concourse import bass2jax as b2j

    b2j.install_neuronx_cc_hook()
    n_cores = len(in_maps)
    partition_name = (nc.partition_id_tensor.name
                      if nc.partition_id_tensor else None)
    in_names, out_names, out_avals, out_shapes = [], [], [], []
    for alloc in nc.m.functions[0].allocations:
        if not isinstance(alloc, mybir.MemoryLocationSet):
            continue
        name = alloc.memorylocations[0].name
        if alloc.kind == "ExternalInput":
            if name != partition_name:
                in_names.append(name)
        elif alloc.kind == "ExternalOutput":
            out_names.append(name)
            shape = tuple(alloc.tensor_shape)
            dtype = mybir.dt.np(alloc.dtype)
            out_avals.append(jax.core.ShapedArray(shape, dtype))
            out_shapes.append((shape, dtype))
    n_params = len(in_names)
    all_names = tuple(in_names + out_names)
    if partition_name is not None:
        all_names = all_names + (partition_name,)

    def _body(*args):
        operands = list(args)
        if partition_name is not None:
            operands.append(b2j.partition_id_tensor())
        for _ in range(reps):
            outs = b2j._bass_exec_p.bind(
                *operands, out_avals=tuple(out_avals), in_names=all_names,
                out_names=tuple(out_names), lowering_input_output_aliases=(),
                sim_require_finite=True, sim_require_nnan=True, nc=nc)
        return tuple(outs)

    devices = jax.devices()[:n_cores]
    mesh = Mesh(np.asarray(devices), ("core",))
    in_specs = (PartitionSpec("core"),) * (n_params + len(out_names))
    out_specs = (PartitionSpec("core"),) * len(out_names)
    fn = jax.jit(shard_map(_body, mesh=mesh, in_specs=in_specs,
                           out_specs=out_specs, check_rep=False),
                 keep_unused=True)
    sh = NamedSharding(mesh, PartitionSpec("core"))
    concat = [np.concatenate([np.asarray(in_maps[c][nm]) for c in
                              range(n_cores)], axis=0) for nm in in_names]
    concat += [np.zeros((shape[0] * n_cores,) + shape[1:], dtype)
               for shape, dtype in out_shapes]
    dev_args = [jax.device_put(a, sh) for a in concat]

    def run():
        outs = fn(*dev_args)
        jax.block_until_ready(outs)
        return outs

    def split_results(outs):
        res = [dict() for _ in range(n_cores)]
        for (shape, dtype), nm, o in zip(out_shapes, out_names, outs):
            o = np.asarray(o)
            for c in range(n_cores):
                res[c][nm] = o[c * shape[0]:(c + 1) * shape[0]]
        return res

    run.fn = fn
    run.dev_args = dev_args
    return run, split_results


if __name__ == "__main__":
    rng = np.random.default_rng(0)
    ins = {
        "x": rng.standard_normal((4, C, 64, 64)).astype(np.float32),
        "gamma": np.ones(C, np.float32), "beta": np.zeros(C, np.float32),
        "wq": (rng.standard_normal((C, C)) / math.sqrt(C)).astype(np.float32),
        "bq": np.zeros(C, np.float32),
        "wk": (rng.standard_normal((C, C)) / math.sqrt(C)).astype(np.float32),
        "bk": np.zeros(C, np.float32),
        "wv": (rng.standard_normal((C, C)) / math.sqrt(C)).astype(np.float32),
        "bv": np.zeros(C, np.float32),
        "wo": (rng.standard_normal((C, C)) / math.sqrt(C)).astype(np.float32),
        "bo": np.zeros(C, np.float32),
    }
    y = kernel(**ins)
    print("kernel ran, output", y.shape, y.dtype)
